# revision 13
# baseline (speedup 1.0000x reference)
"""ContextualConv2d Trainium2 kernel.

out = conv2d(x, weight, pad=1) + MLP(c)[:, :, None, None] + bias[None, :, None, None]

Sharding: data-parallel over batch N=16 -> 2 samples per core on 8 cores.
weight / MLP params replicated. Conv is an implicit GEMM: for each of the
9 filter taps and 2 input-channel chunks, a [K=128ci, M=128co] x
[K=128ci, N=512spatial] matmul accumulates into PSUM. fp32 data is fed to
the PE in float32r mode (full-rate) -- accumulation is fp32 in PSUM.
"""

import numpy as np

import concourse.bass as bass
import concourse.mybir as mybir
import concourse.tile as tile
from concourse import bacc, bass_utils

# Problem shapes (hardcoded; kernel.py must be self-contained).
N, C_IN, H, W = 16, 256, 64, 64
C_OUT, KH, KW = 256, 3, 3
C_DIM, H_DIM = 128, 512

N_CORES = 8
N_LOC = N // N_CORES          # 2 samples per core
P = 128
CIO = C_IN // P               # 2 input-channel chunks
COO = C_OUT // P              # 2 output-channel chunks
HO = H_DIM // P               # 4 hidden chunks
HP, WP = H + 2, W + 2         # padded image 66x66
RB = 8                        # output rows per matmul tile
S_TILES = H // RB             # 8 spatial tiles per (n, coo)
FREE = RB * W                 # 512 = matmul free dim (max for fp32)

# Set by test harness for profiling; grading path uses defaults.
TRACE = False
LAST_RESULTS = None

_BUILT = None


def _build():
    """Build the SPMD Bass program (one NEFF, runs on all 8 cores)."""
    f32 = mybir.dt.float32
    f32r = mybir.dt.float32r
    AF = mybir.ActivationFunctionType

    nc = bacc.Bacc("TRN2", target_bir_lowering=False, debug=False,
                   num_devices=N_CORES)

    # Conv operands are declared float32r (same bits as fp32): the PE's
    # full-rate fp32 mode. The verifier requires the whole producer chain
    # to be fp32r-typed. x arrives zero-padded to 66x66 from the host so
    # every tap is a full-size matmul with a strided ifmap AP.
    x_d = nc.dram_tensor("x", [N_LOC, C_IN, HP, WP], f32r, kind="ExternalInput").ap()
    # weight pre-transposed on host to [ci, kh*kw*co]
    wt_d = nc.dram_tensor("wt", [C_IN, KH * KW * C_OUT], f32r, kind="ExternalInput").ap()
    # c.T slice for this core's samples: [c_dim, n_loc]
    ct_d = nc.dram_tensor("ct", [C_DIM, N_LOC], f32, kind="ExternalInput").ap()
    w1t_d = nc.dram_tensor("w1t", [C_DIM, H_DIM], f32, kind="ExternalInput").ap()
    w2t_d = nc.dram_tensor("w2t", [H_DIM, C_OUT], f32, kind="ExternalInput").ap()
    b1_d = nc.dram_tensor("b1c", [P, HO], f32, kind="ExternalInput").ap()
    # bb = bias + b2, chunked [P, COO]
    bb_d = nc.dram_tensor("bb", [P, COO], f32, kind="ExternalInput").ap()
    y_d = nc.dram_tensor("y", [N_LOC, C_OUT, H, W], f32, kind="ExternalOutput").ap()

    with tile.TileContext(nc) as tc:
        with (
            tc.tile_pool(name="const", bufs=1) as const,
            tc.tile_pool(name="xpad", bufs=2) as xpad_pool,
            tc.tile_pool(name="outp", bufs=4) as out_pool,
            tc.tile_pool(name="psum_mlp", bufs=2, space="PSUM") as psum_mlp,
            tc.tile_pool(name="psum_conv", bufs=6, space="PSUM") as psum_conv,
        ):
            # ---- constants ----
            w_sb = const.tile([P, CIO, KH * KW * C_OUT], f32r)
            nc.sync.dma_start(w_sb[:], wt_d.rearrange("(c p) f -> p c f", p=P))

            ct_sb = const.tile([C_DIM, N_LOC], f32)
            nc.sync.dma_start(ct_sb[:], ct_d)
            w1t_sb = const.tile([C_DIM, H_DIM], f32)
            nc.sync.dma_start(w1t_sb[:], w1t_d)
            w2t_sb = const.tile([P, HO, C_OUT], f32)
            nc.sync.dma_start(w2t_sb[:], w2t_d.rearrange("(o p) f -> p o f", p=P))
            b1_sb = const.tile([P, HO], f32)
            nc.sync.dma_start(b1_sb[:], b1_d)
            bb_sb = const.tile([P, COO], f32)
            nc.sync.dma_start(bb_sb[:], bb_d)

            # ---- context MLP: ctxb[co, nl] = w2 @ relu(w1 @ c.T + b1) + bias + b2
            h_sb = const.tile([P, HO, N_LOC], f32)
            for ho in range(HO):
                ps = psum_mlp.tile([P, N_LOC], f32, tag="mlp")
                nc.tensor.matmul(ps[:], w1t_sb[:, ho * P:(ho + 1) * P], ct_sb[:],
                                 start=True, stop=True)
                nc.scalar.activation(h_sb[:, ho, :], ps[:], AF.Relu,
                                     bias=b1_sb[:, ho:ho + 1])
            ctxb_sb = const.tile([P, COO, N_LOC], f32)
            for coo in range(COO):
                ps = psum_mlp.tile([P, N_LOC], f32, tag="mlp")
                for ho in range(HO):
                    nc.tensor.matmul(ps[:], w2t_sb[:, ho, coo * P:(coo + 1) * P],
                                     h_sb[:, ho, :],
                                     start=(ho == 0), stop=(ho == HO - 1))
                nc.scalar.activation(ctxb_sb[:, coo, :], ps[:], AF.Identity,
                                     bias=bb_sb[:, coo:coo + 1])

            # ---- conv ----
            for nl in range(N_LOC):
                xs = xpad_pool.tile([P, CIO, HP * WP], f32r, tag="xp")
                xv = xs.rearrange("p c (h w) -> p c h w", w=WP)
                for cio in range(CIO):
                    nc.sync.dma_start(xv[:, cio], x_d[nl, cio * P:(cio + 1) * P])

                for coo in range(COO):
                    for s in range(S_TILES):
                        r0 = s * RB
                        ps = psum_conv.tile([P, FREE], f32, tag="cps")
                        idx = 0
                        for cio in range(CIO):
                            for kh in range(KH):
                                for kw in range(KW):
                                    lw = w_sb[:, cio,
                                              (kh * KW + kw) * C_OUT + coo * P:
                                              (kh * KW + kw) * C_OUT + coo * P + P]
                                    rhs = xv[:, cio, r0 + kh:r0 + kh + RB,
                                             kw:kw + W]
                                    nc.tensor.matmul(
                                        ps[:], lw, rhs,
                                        start=(idx == 0),
                                        stop=(idx == CIO * KH * KW - 1))
                                    idx += 1
                        ot = out_pool.tile([P, FREE], f32, tag="ot")
                        nc.scalar.activation(ot[:], ps[:], AF.Identity,
                                             bias=ctxb_sb[:, coo, nl:nl + 1])
                        nc.sync.dma_start(
                            y_d[nl, coo * P:(coo + 1) * P, s * RB:(s + 1) * RB, :],
                            ot.rearrange("p (h w) -> p h w", w=W))
    nc.compile()
    return nc


def kernel(**inputs):
    global _BUILT, LAST_RESULTS
    x = np.asarray(inputs["x"], dtype=np.float32)
    # zero-pad the image on the host: conv(pad=1) taps become uniform
    # full-size matmuls over a strided view of the padded image
    xp = np.zeros((N, C_IN, HP, WP), dtype=np.float32)
    xp[:, :, 1:1 + H, 1:1 + W] = x
    c = np.asarray(inputs["c"], dtype=np.float32)
    weight = np.asarray(inputs["weight"], dtype=np.float32)
    bias = np.asarray(inputs["bias"], dtype=np.float32)
    w1 = np.asarray(inputs["w1"], dtype=np.float32)
    b1 = np.asarray(inputs["b1"], dtype=np.float32)
    w2 = np.asarray(inputs["w2"], dtype=np.float32)
    b2 = np.asarray(inputs["b2"], dtype=np.float32)

    # host-side layout prep (replicated operands)
    wt = np.ascontiguousarray(weight.transpose(1, 2, 3, 0)).reshape(
        C_IN, KH * KW * C_OUT)
    ct_full = np.ascontiguousarray(c.T)                    # [c_dim, N]
    w1t = np.ascontiguousarray(w1.T)                       # [c_dim, h_dim]
    w2t = np.ascontiguousarray(w2.T)                       # [h_dim, c_out]
    b1c = np.ascontiguousarray(b1.reshape(HO, P).T)        # [P, HO]
    bb = np.ascontiguousarray((bias + b2).reshape(COO, P).T)  # [P, COO]

    if _BUILT is None:
        _BUILT = _build()
    nc = _BUILT

    in_maps = []
    for k in range(N_CORES):
        n0 = k * N_LOC
        in_maps.append({
            "x": np.ascontiguousarray(xp[n0:n0 + N_LOC]),
            "wt": wt,
            "ct": np.ascontiguousarray(ct_full[:, n0:n0 + N_LOC]),
            "w1t": w1t,
            "w2t": w2t,
            "b1c": b1c,
            "bb": bb,
        })

    res = bass_utils.run_bass_kernel_spmd(
        nc, in_maps, core_ids=list(range(N_CORES)), trace=TRACE)
    LAST_RESULTS = res
    return np.concatenate([r["y"] for r in res.results], axis=0)


# revision 15
# speedup vs baseline: 1.0117x; 1.0117x over previous
"""ContextualConv2d Trainium2 kernel.

out = conv2d(x, weight, pad=1) + MLP(c)[:, :, None, None] + bias[None, :, None, None]

Sharding: data-parallel over batch N=16 -> 2 samples per core on 8 cores.
weight / MLP params replicated. Conv is an implicit GEMM: for each of the
9 filter taps and 2 input-channel chunks, a [K=128ci, M=128co] x
[K=128ci, N=512spatial] matmul accumulates into PSUM. fp32 data is fed to
the PE in float32r mode (full-rate) -- accumulation is fp32 in PSUM.
"""

import numpy as np

import concourse.bass as bass
import concourse.mybir as mybir
import concourse.tile as tile
from concourse import bacc, bass_utils

# Problem shapes (hardcoded; kernel.py must be self-contained).
N, C_IN, H, W = 16, 256, 64, 64
C_OUT, KH, KW = 256, 3, 3
C_DIM, H_DIM = 128, 512

N_CORES = 8
N_LOC = N // N_CORES          # 2 samples per core
P = 128
CIO = C_IN // P               # 2 input-channel chunks
COO = C_OUT // P              # 2 output-channel chunks
HO = H_DIM // P               # 4 hidden chunks
HP, WP = H + 2, W + 2         # padded image 66x66
RB = 8                        # output rows per matmul tile
S_TILES = H // RB             # 8 spatial tiles per (n, coo)
FREE = RB * W                 # 512 = matmul free dim (max for fp32)
KWF = KH * KW * C_OUT         # weight free size per ci chunk (2304)

# packed MLP-const layout (single DMA): per partition p the columns are
# [ct(2) | w1t(512) | b1(4) | bb(2) | w2t(1024)]
MC_CT, MC_W1, MC_B1, MC_BB, MC_W2 = 0, 2, 514, 518, 520
MC_TOT = 520 + HO * C_OUT     # 1544

# Set by test harness for profiling; grading path uses defaults.
TRACE = False
LAST_RESULTS = None

_BUILT = None


def _build():
    """Build the SPMD Bass program (one NEFF, runs on all 8 cores)."""
    f32 = mybir.dt.float32
    f32r = mybir.dt.float32r
    AF = mybir.ActivationFunctionType

    nc = bacc.Bacc("TRN2", target_bir_lowering=False, debug=False,
                   num_devices=N_CORES)

    # Conv operands are declared float32r (same bits as fp32): the PE's
    # full-rate fp32 mode. The verifier requires the whole producer chain
    # to be fp32r-typed. x arrives zero-padded to 66x66 from the host so
    # every tap is a full-size matmul with a strided ifmap AP.
    x_d = nc.dram_tensor("x", [N_LOC, C_IN, HP, WP], f32r, kind="ExternalInput").ap()
    # weight pre-transposed on host to [ci, kh*kw*co]
    wt_d = nc.dram_tensor("wt", [C_IN, KWF], f32r, kind="ExternalInput").ap()
    # packed MLP constants (ct slice differs per core)
    mc_d = nc.dram_tensor("mc", [P, MC_TOT], f32, kind="ExternalInput").ap()
    y_d = nc.dram_tensor("y", [N_LOC, C_OUT, H, W], f32, kind="ExternalOutput").ap()

    with tile.TileContext(nc) as tc:
        with (
            tc.tile_pool(name="const", bufs=1) as const,
            tc.tile_pool(name="xpad", bufs=2) as xpad_pool,
            tc.tile_pool(name="outp", bufs=4) as out_pool,
            tc.tile_pool(name="psum_mlp", bufs=2, space="PSUM") as psum_mlp,
            tc.tile_pool(name="psum_conv", bufs=6, space="PSUM") as psum_conv,
        ):
            # ---- constant DMAs (issue order matters: small MLP pack first,
            # then conv weights split per ci-chunk across DMA lanes) ----
            mc_sb = const.tile([P, MC_TOT], f32)
            nc.sync.dma_start(mc_sb[:], mc_d)
            wr = wt_d.rearrange("(c p) f -> c p f", p=P)
            w_sb = []
            for cio in range(CIO):
                wt_t = const.tile([P, KWF], f32r, tag=f"w{cio}")
                nc.sync.dma_start(wt_t[:], wr[cio])
                w_sb.append(wt_t)

            ct_sb = mc_sb[:, MC_CT:MC_CT + N_LOC]
            w2v = mc_sb[:, MC_W2:MC_W2 + HO * C_OUT].rearrange(
                "p (o f) -> p o f", f=C_OUT)

            # ---- context MLP: ctxb[co, nl] = w2 @ relu(w1 @ c.T + b1) + bias + b2
            h_sb = const.tile([P, HO, N_LOC], f32)
            for ho in range(HO):
                ps = psum_mlp.tile([P, N_LOC], f32, tag="mlp")
                nc.tensor.matmul(ps[:],
                                 mc_sb[:, MC_W1 + ho * P:MC_W1 + (ho + 1) * P],
                                 ct_sb, start=True, stop=True)
                nc.scalar.activation(h_sb[:, ho, :], ps[:], AF.Relu,
                                     bias=mc_sb[:, MC_B1 + ho:MC_B1 + ho + 1])
            ctxb_sb = const.tile([P, COO, N_LOC], f32)
            for coo in range(COO):
                ps = psum_mlp.tile([P, N_LOC], f32, tag="mlp")
                for ho in range(HO):
                    nc.tensor.matmul(ps[:], w2v[:, ho, coo * P:(coo + 1) * P],
                                     h_sb[:, ho, :],
                                     start=(ho == 0), stop=(ho == HO - 1))
                nc.scalar.activation(ctxb_sb[:, coo, :], ps[:], AF.Identity,
                                     bias=mc_sb[:, MC_BB + coo:MC_BB + coo + 1])

            # ---- conv ----
            for nl in range(N_LOC):
                xs = xpad_pool.tile([P, CIO, HP * WP], f32r, tag="xp")
                xv = xs.rearrange("p c (h w) -> p c h w", w=WP)
                for cio in range(CIO):
                    # split the image DMA in row halves -> two DMA lanes
                    nc.sync.dma_start(xv[:, cio, :HP // 2, :],
                                      x_d[nl, cio * P:(cio + 1) * P, :HP // 2, :])
                    nc.sync.dma_start(xv[:, cio, HP // 2:, :],
                                      x_d[nl, cio * P:(cio + 1) * P, HP // 2:, :])

                for coo in range(COO):
                    for s in range(S_TILES):
                        r0 = s * RB
                        ps = psum_conv.tile([P, FREE], f32, tag="cps")
                        idx = 0
                        for cio in range(CIO):
                            for kh in range(KH):
                                for kw in range(KW):
                                    lw = w_sb[cio][:,
                                                   (kh * KW + kw) * C_OUT + coo * P:
                                                   (kh * KW + kw) * C_OUT + coo * P + P]
                                    rhs = xv[:, cio, r0 + kh:r0 + kh + RB,
                                             kw:kw + W]
                                    nc.tensor.matmul(
                                        ps[:], lw, rhs,
                                        start=(idx == 0),
                                        stop=(idx == CIO * KH * KW - 1))
                                    idx += 1
                        ot = out_pool.tile([P, FREE], f32, tag="ot")
                        nc.scalar.activation(ot[:], ps[:], AF.Identity,
                                             bias=ctxb_sb[:, coo, nl:nl + 1])
                        nc.sync.dma_start(
                            y_d[nl, coo * P:(coo + 1) * P, s * RB:(s + 1) * RB, :],
                            ot.rearrange("p (h w) -> p h w", w=W))
    nc.compile()
    return nc


def kernel(**inputs):
    global _BUILT, LAST_RESULTS
    x = np.asarray(inputs["x"], dtype=np.float32)
    # zero-pad the image on the host: conv(pad=1) taps become uniform
    # full-size matmuls over a strided view of the padded image
    xp = np.zeros((N, C_IN, HP, WP), dtype=np.float32)
    xp[:, :, 1:1 + H, 1:1 + W] = x
    c = np.asarray(inputs["c"], dtype=np.float32)
    weight = np.asarray(inputs["weight"], dtype=np.float32)
    bias = np.asarray(inputs["bias"], dtype=np.float32)
    w1 = np.asarray(inputs["w1"], dtype=np.float32)
    b1 = np.asarray(inputs["b1"], dtype=np.float32)
    w2 = np.asarray(inputs["w2"], dtype=np.float32)
    b2 = np.asarray(inputs["b2"], dtype=np.float32)

    # host-side layout prep (replicated operands)
    wt = np.ascontiguousarray(weight.transpose(1, 2, 3, 0)).reshape(C_IN, KWF)
    ct_full = np.ascontiguousarray(c.T)                    # [c_dim, N]

    mc_base = np.empty((P, MC_TOT), dtype=np.float32)
    mc_base[:, MC_W1:MC_W1 + H_DIM] = w1.T
    mc_base[:, MC_B1:MC_B1 + HO] = b1.reshape(HO, P).T
    mc_base[:, MC_BB:MC_BB + COO] = (bias + b2).reshape(COO, P).T
    mc_base[:, MC_W2:] = (w2.T.reshape(HO, P, C_OUT)
                          .transpose(1, 0, 2).reshape(P, HO * C_OUT))

    if _BUILT is None:
        _BUILT = _build()
    nc = _BUILT

    in_maps = []
    for k in range(N_CORES):
        n0 = k * N_LOC
        mc = mc_base.copy()
        mc[:, MC_CT:MC_CT + N_LOC] = ct_full[:, n0:n0 + N_LOC]
        in_maps.append({
            "x": np.ascontiguousarray(xp[n0:n0 + N_LOC]),
            "wt": wt,
            "mc": mc,
        })

    res = bass_utils.run_bass_kernel_spmd(
        nc, in_maps, core_ids=list(range(N_CORES)), trace=TRACE)
    LAST_RESULTS = res
    return np.concatenate([r["y"] for r in res.results], axis=0)
